# revision 16
# baseline (speedup 1.0000x reference)
"""Classical self-attention (head-summed scores) on 8 trn2 NeuronCores.

Math (per batch b):
    Q = x Wq; K = x Wk; V = x Wv          (W_qkv split columns 3x1024)
    S = Q K^T / 8   (full-E contraction: heads+dims summed)
    P = softmax(S, axis=-1)
    out = (P V) W_out + b_out

Sharding: 8 cores = (4 batches) x (2 query-halves). Each core gets its
batch's x rotated so its 1024 query rows come first; keys are the full
2048 rows (key order is irrelevant to the result). K/V projections are
duplicated between the 2 cores of a batch; no collectives needed.

v2 design (vs v1):
  - Inputs host-converted to bf16: halves DMA and SBUF, same PE rate
    (1.0 cycles/row) as fp32r in the TRN2 cost model.
  - x^T produced by DMA-transpose (XBAR) straight from DRAM: no PE
    transposes, no x staging in SBUF. Same-type DMAs are grouped (the
    tile framework serializes DMA streams at transpose<->copy mode
    switches), with Wk loads first so the warmup K matmuls can start
    as soon as the first x^T tile lands.
  - K^T, V, Q^T, P all SBUF-resident in bf16 -- no DRAM staging.
  - All matmuls emit <=512-element moving patterns (ISA limit).
  - Warmup: first 4 K^T chunks run f-outer so PE streams while x^T
    tiles are still arriving from DRAM.
  - Softmax skips max-subtraction (scores ~ N(0,16) after 1/8 scale);
    1/rowsum is deferred to the output projection.
  - ACT does only the exps (and half the O^T copies); DVE does the
    PSUM->SBUF copies; Pool does the bias adds. Output tail is split
    into 512-wide halves to shorten the post-PE drain.
"""

import sys

sys.path.insert(0, "/opt/trn_rl_repo")

import numpy as np

import concourse.bass as bass
import concourse.mybir as mybir
import concourse.tile as tile
from concourse import bacc

B, N, E = 4, 2048, 1024
NQ = N // 2          # query rows per core
P = 128              # partitions
FT = E // P          # 8 feature (contraction) tiles
ET = E // P          # 8 embed tiles
MT = N // P          # 16 key tiles
QT = NQ // P         # 8 query tiles
HKEY = N // 2        # 1024 keys per half (x^T transpose granularity)
MMF = 512            # max moving elements per matmul instruction
F32 = mybir.dt.float32
BF16 = mybir.dt.bfloat16
EXP = mybir.ActivationFunctionType.Exp


def build_program():
    nc = bacc.Bacc("TRN2", target_bir_lowering=False, debug=False)
    x = nc.dram_tensor("x", [N, E], BF16, kind="ExternalInput").ap()
    wqkv = nc.dram_tensor("wqkv", [E, 3 * E], BF16, kind="ExternalInput").ap()
    wout = nc.dram_tensor("wout", [E, E], BF16, kind="ExternalInput").ap()
    bout = nc.dram_tensor("bout", [E], F32, kind="ExternalInput").ap()
    y = nc.dram_tensor("y", [NQ, E], F32, kind="ExternalOutput").ap()

    with tile.TileContext(nc) as tc:
        _body(nc, tc, x, wqkv, wout, bout, y)
    nc.compile()
    return nc


def _mm(nc, out, lhsT, rhs, start, stop, width):
    """Accumulating matmul split into <=512-wide moving chunks."""
    for c0 in range(0, width, MMF):
        nc.tensor.matmul(out[:, c0:c0 + MMF], lhsT, rhs[:, c0:c0 + MMF],
                         start=start, stop=stop)


def _body(nc, tc, x, wqkv, wout, bout, y):
    # ---- persistent SBUF residents (right side) --------------------------
    wop = tc.alloc_tile_pool(name="wo", bufs=1, side="right")
    wo = [wop.tile([P, E], BF16, name=f"wo{e}", tag=f"wo{e}") for e in range(ET)]
    bo_b = wop.tile([P, E], F32, name="bo_b", tag="bo_b")

    kqp = tc.alloc_tile_pool(name="kq", bufs=1, side="right")
    kT = [kqp.tile([P, N], BF16, name=f"kT{e}", tag=f"kT{e}") for e in range(ET)]
    qT = [kqp.tile([P, NQ], BF16, name=f"qT{e}", tag=f"qT{e}") for e in range(ET)]

    vp = tc.alloc_tile_pool(name="vp", bufs=1, side="right")
    v = [vp.tile([P, E], BF16, name=f"v{m}", tag=f"v{m}") for m in range(MT)]

    smp = tc.alloc_tile_pool(name="small", bufs=1, side="right")
    ones = smp.tile([P, 1], BF16, name="ones", tag="ones")
    sums_acc = smp.tile([P, QT], F32, name="sums_acc", tag="sums_acc")
    recip = smp.tile([P, QT], F32, name="recip", tag="recip")
    actwarm = smp.tile([P, 1], F32, name="actwarm", tag="actwarm")

    # ---- phase 1: load + project ----------------------------------------
    wp = tc.alloc_tile_pool(name="wqkv", bufs=1)
    wk = [wp.tile([P, E], BF16, name=f"wk{f}", tag=f"wk{f}") for f in range(FT)]
    wq = [wp.tile([P, E], BF16, name=f"wq{f}", tag=f"wq{f}") for f in range(FT)]
    wv = [wp.tile([P, E], BF16, name=f"wv{f}", tag=f"wv{f}") for f in range(FT)]

    xTp = tc.alloc_tile_pool(name="xT", bufs=1)
    # xT[f][h]: [128, 1024] = transpose of x[h*1024:(h+1)*1024, f*128:(f+1)*128]
    xT = [[xTp.tile([P, HKEY], BF16, name=f"xT{f}_{h}", tag=f"xT{f}_{h}")
           for h in range(2)] for f in range(FT)]

    # DMA order (one queue; transfers execute in this order). Wk0-3 first
    # so the warmup matmuls are gated only by the x^T transposes, which
    # land one per ~0.9us -- slower than the 1.7us/round the warmup
    # consumes; wk4-7 slot in after half 0 (0.73us/tile, still ahead of
    # the rounds that need them). Same-type DMAs stay contiguous: a
    # transpose<->copy switch serializes the two transfers back-to-back
    # (XBAR mode drain).
    for f in range(4):
        nc.sync.dma_start(out=wk[f], in_=wqkv[f * P:(f + 1) * P, E:2 * E])
    for f in range(FT):
        nc.sync.dma_start_transpose(
            out=xT[f][0], in_=x[0:HKEY, f * P:(f + 1) * P])
    for f in range(4, FT):
        nc.sync.dma_start(out=wk[f], in_=wqkv[f * P:(f + 1) * P, E:2 * E])
    for f in range(FT):
        nc.sync.dma_start_transpose(
            out=xT[f][1], in_=x[HKEY:N, f * P:(f + 1) * P])
    for f in range(FT):
        nc.sync.dma_start(out=wq[f], in_=wqkv[f * P:(f + 1) * P, 0:E])
    for f in range(FT):
        nc.sync.dma_start(out=wv[f], in_=wqkv[f * P:(f + 1) * P, 2 * E:3 * E])
    for e in range(ET):
        nc.sync.dma_start(out=wo[e], in_=wout[e * P:(e + 1) * P, :])
    bout_bcast = bass.AP(tensor=bout.tensor, offset=0, ap=[[0, P], [1, E]])
    nc.sync.dma_start(out=bo_b, in_=bout_bcast)
    nc.vector.memset(ones, 1.0)
    # Preload the Exp act-table while ACT is idle: the compiler inserts
    # LoadActFuncSet (1.3us) before the first Exp; without this it lands
    # on the critical path at the scores boundary. ACT runs ONLY Exp --
    # aux copies go to Pool/DVE so no Copy<->Exp table reloads occur.
    nc.scalar.activation(actwarm, ones, EXP, scale=0.125)

    with tc.tile_pool(name="pjps", bufs=4, space="PSUM") as pjp:
        # Warmup: first 4 K chunks (e=0..3, keys half 0) f-outer.
        warm = [pjp.tile([P, HKEY], F32, name=f"pw{e}", tag="pj")
                for e in range(4)]
        for f in range(FT):
            for e in range(4):
                _mm(nc, warm[e], wk[f][:, e * P:(e + 1) * P], xT[f][0],
                    start=(f == 0), stop=(f == FT - 1), width=HKEY)
        for e in range(4):
            nc.vector.tensor_copy(kT[e][:, 0:HKEY], warm[e])

        # Remaining K chunks (f-inner), then Q, then V.
        for (e, h) in [(e, h) for h in range(2) for e in range(ET)
                       if not (h == 0 and e < 4)]:
            ps = pjp.tile([P, HKEY], F32, name="pjk", tag="pj")
            for f in range(FT):
                _mm(nc, ps, wk[f][:, e * P:(e + 1) * P], xT[f][h],
                    start=(f == 0), stop=(f == FT - 1), width=HKEY)
            nc.vector.tensor_copy(kT[e][:, h * HKEY:(h + 1) * HKEY], ps)

        for e in range(ET):
            ps = pjp.tile([P, NQ], F32, name="pjq", tag="pj")
            for f in range(FT):
                _mm(nc, ps, wq[f][:, e * P:(e + 1) * P], xT[f][0],
                    start=(f == 0), stop=(f == FT - 1), width=NQ)
            nc.vector.tensor_copy(qT[e], ps)

        for m in range(MT):
            h, mm_ = divmod(m, ET)
            ps = pjp.tile([P, E], F32, name="pjv", tag="pj")
            for f in range(FT):
                _mm(nc, ps, xT[f][h][:, mm_ * P:(mm_ + 1) * P], wv[f],
                    start=(f == 0), stop=(f == FT - 1), width=E)
            if m < MT - 1:
                nc.vector.tensor_copy(v[m], ps)
            else:
                # Last projection drain gates the scores PSUM pool: split
                # across DVE+Pool so it clears in half the time.
                nc.vector.tensor_copy(v[m][:, 0:E // 2], ps[:, 0:E // 2])
                nc.gpsimd.tensor_copy(v[m][:, E // 2:E], ps[:, E // 2:E])

    xTp.release()
    wp.release()

    # ---- phase 2: scores + exp + row-sums --------------------------------
    pp = tc.alloc_tile_pool(name="pp", bufs=1)
    p_tiles = [pp.tile([P, NQ], BF16, name=f"p{m}", tag=f"p{m}")
               for m in range(MT)]

    with tc.tile_pool(name="sps", bufs=3, space="PSUM") as sp, \
         tc.tile_pool(name="sumps", bufs=2, space="PSUM") as sumsp:
        for m in range(MT):
            s = sp.tile([P, NQ], F32, name="s", tag="s")
            for e in range(ET):
                _mm(nc, s, kT[e][:, m * P:(m + 1) * P], qT[e],
                    start=(e == 0), stop=(e == ET - 1), width=NQ)
            if m < MT - 1:
                nc.scalar.activation(p_tiles[m], s, EXP, scale=0.125)
            else:
                # Last exp gates the PV PSUM pool: split into halves.
                for hh in range(2):
                    sl = slice(hh * (NQ // 2), (hh + 1) * (NQ // 2))
                    nc.scalar.activation(p_tiles[m][:, sl], s[:, sl], EXP,
                                         scale=0.125)
            # Row-sum the PREVIOUS tile's exp while this tile's S matmuls
            # occupy PE, so PE never waits on ACT.
            if m > 0:
                _row_sums(nc, p_tiles[m - 1], sumsp, ones, sums_acc,
                          first=(m == 1))
        _row_sums(nc, p_tiles[MT - 1], sumsp, ones, sums_acc, first=False)
        nc.vector.reciprocal(recip, sums_acc)

    # ---- phase 3: O^T = sum_m V[m]^T P^T[m] ------------------------------
    oTp = tc.alloc_tile_pool(name="oTp", bufs=1)
    oT = [oTp.tile([P, NQ], BF16, name=f"oT{e}", tag=f"oT{e}")
          for e in range(ET)]
    EG = ET // 2
    with tc.tile_pool(name="ops", bufs=1, space="PSUM") as opp:
        for g in range(2):
            o_ps = [opp.tile([P, NQ], F32, name=f"o{j}", tag=f"o{j}")
                    for j in range(EG)]
            for m in range(MT):
                for j in range(EG):
                    e = g * EG + j
                    _mm(nc, o_ps[j], v[m][:, e * P:(e + 1) * P], p_tiles[m],
                        start=(m == 0), stop=(m == MT - 1), width=NQ)
            # Alternate copy engines (and split halves on the last group)
            # so the next phase gets PSUM banks back fast.
            for j in range(EG):
                e = g * EG + j
                if g == 0:
                    if j % 2 == 0:
                        nc.vector.tensor_copy(oT[e], o_ps[j])
                    else:
                        nc.gpsimd.tensor_copy(oT[e], o_ps[j])
                else:
                    h2 = NQ // 2
                    nc.vector.tensor_copy(oT[e][:, 0:h2], o_ps[j][:, 0:h2])
                    nc.gpsimd.tensor_copy(oT[e][:, h2:NQ], o_ps[j][:, h2:NQ])

    # ---- phase 4: y = (O^T.T W_out) * recip + b_out ----------------------
    H2 = E // 2
    with tc.tile_pool(name="ysb", bufs=3) as ysp, \
         tc.tile_pool(name="yps", bufs=2, space="PSUM") as ypp:
        for nqt in range(QT):
            yps = ypp.tile([P, E], F32, name="yps", tag="yps")
            for e in range(ET):
                _mm(nc, yps, oT[e][:, nqt * P:(nqt + 1) * P], wo[e],
                    start=(e == 0), stop=(e == ET - 1), width=E)
            ysb = ysp.tile([P, E], F32, name="ysb", tag="ysb")
            # Fused (yps * recip) + b_out on DVE, split so the drain after
            # the last matmul is short; the final chunk goes 4-way with
            # DVE/Pool alternating to squeeze the tail further.
            nsplit = 4 if nqt == QT - 1 else 2
            w = E // nsplit
            for hh in range(nsplit):
                sl = slice(hh * w, (hh + 1) * w)
                eng = nc.vector if hh % 2 == 0 or nsplit == 2 else nc.gpsimd
                eng.scalar_tensor_tensor(
                    out=ysb[:, sl], in0=yps[:, sl],
                    scalar=recip[:, nqt:nqt + 1], in1=bo_b[:, sl],
                    op0=mybir.AluOpType.mult, op1=mybir.AluOpType.add)
                nc.sync.dma_start(out=y[nqt * P:(nqt + 1) * P, sl],
                                  in_=ysb[:, sl])

    oTp.release()
    pp.release()
    smp.release()
    vp.release()
    kqp.release()
    wop.release()


def _row_sums(nc, p, sumsp, ones, sums_acc, first):
    sums_m = sumsp.tile([P, QT], F32, name="sums_m", tag="sums_m")
    for q in range(QT):
        nc.tensor.matmul(sums_m[:, q:q + 1], p[:, q * P:(q + 1) * P], ones,
                         start=True, stop=True)
    if first:
        nc.vector.tensor_copy(sums_acc, sums_m)
    else:
        nc.vector.tensor_tensor(out=sums_acc, in0=sums_acc,
                                in1=sums_m, op=mybir.AluOpType.add)


_NC_CACHE = None


def _get_program():
    global _NC_CACHE
    if _NC_CACHE is None:
        _NC_CACHE = build_program()
    return _NC_CACHE


def kernel(x, W_qkv, W_out, b_out):
    import ml_dtypes
    from concourse.bass_utils import run_bass_kernel_spmd

    bf16 = ml_dtypes.bfloat16
    x = np.asarray(x, dtype=np.float32).astype(bf16)
    W_qkv = np.asarray(W_qkv, dtype=np.float32).astype(bf16)
    W_out = np.asarray(W_out, dtype=np.float32).astype(bf16)
    b_out = np.asarray(b_out, dtype=np.float32)

    nc = _get_program()
    in_maps = []
    for c in range(8):
        b, half = divmod(c, 2)
        xb = x[b]
        xrot = np.ascontiguousarray(
            np.concatenate([xb[half * NQ:], xb[:half * NQ]], axis=0))
        in_maps.append({"x": xrot, "wqkv": W_qkv, "wout": W_out,
                       "bout": b_out})
    res = run_bass_kernel_spmd(nc, in_maps, list(range(8)))
    out = np.empty((B, N, E), dtype=np.float32)
    for c in range(8):
        b, half = divmod(c, 2)
        out[b, half * NQ:(half + 1) * NQ] = res.results[c]["y"]
    return out


# revision 19
# speedup vs baseline: 1.0281x; 1.0281x over previous
"""Classical self-attention (head-summed scores) on 8 trn2 NeuronCores.

Math (per batch b):
    Q = x Wq; K = x Wk; V = x Wv          (W_qkv split columns 3x1024)
    S = Q K^T / 8   (full-E contraction: heads+dims summed)
    P = softmax(S, axis=-1)
    out = (P V) W_out + b_out

Sharding: 8 cores = (4 batches) x (2 query-halves). Each core gets its
batch's x rotated so its 1024 query rows come first; keys are the full
2048 rows (key order is irrelevant to the result). K/V projections are
duplicated between the 2 cores of a batch; no collectives needed.

Design notes:
  - Inputs host-converted to bf16: halves DMA and SBUF, same PE rate
    (1.0 cycles/row) as fp32r on TRN2.
  - x^T produced by DMA-transpose (XBAR) straight from DRAM. Same-type
    DMAs are grouped: a transpose<->copy mode switch drains the DMA
    pipe, serializing the two transfers. Wk0-3 load first so the
    f-outer warmup K matmuls start on the first x^T tiles.
  - K^T, V, Q^T, P all SBUF-resident in bf16 -- no DRAM staging.
  - All matmuls emit <=512-element moving patterns (ISA limit).
  - ONE PSUM pool with 4 rotating [128,1024] slots (ps0-3) spans all
    phases; the slot schedule is chosen so no phase boundary ever
    waits on a freshly-written slot (no pool-release stalls).
  - Softmax skips max-subtraction (scores ~ N(0,16) after 1/8 scale);
    1/rowsum deferred to the output projection. Row-sums of the last
    P tile + the reciprocal are folded into the first PV group.
  - ACT runs ONLY Exp (table preloaded at t=0 -- a Copy<->Exp switch
    reloads the 1.3us act table); aux PSUM->SBUF copies go to DVE and
    Pool. Output drain is fused mul+add on DVE, split 4-way at the end.
"""

import sys

sys.path.insert(0, "/opt/trn_rl_repo")

import numpy as np

import concourse.bass as bass
import concourse.mybir as mybir
import concourse.tile as tile
from concourse import bacc

B, N, E = 4, 2048, 1024
NQ = N // 2          # query rows per core
P = 128              # partitions
FT = E // P          # 8 feature (contraction) tiles
ET = E // P          # 8 embed tiles
MT = N // P          # 16 key tiles
QT = NQ // P         # 8 query tiles
HKEY = N // 2        # 1024 keys per half (x^T transpose granularity)
MMF = 512            # max moving elements per matmul instruction
F32 = mybir.dt.float32
BF16 = mybir.dt.bfloat16
EXP = mybir.ActivationFunctionType.Exp


def build_program():
    nc = bacc.Bacc("TRN2", target_bir_lowering=False, debug=False)
    x = nc.dram_tensor("x", [N, E], BF16, kind="ExternalInput").ap()
    wqkv = nc.dram_tensor("wqkv", [E, 3 * E], BF16, kind="ExternalInput").ap()
    wout = nc.dram_tensor("wout", [E, E], BF16, kind="ExternalInput").ap()
    bout = nc.dram_tensor("bout", [E], F32, kind="ExternalInput").ap()
    y = nc.dram_tensor("y", [NQ, E], F32, kind="ExternalOutput").ap()

    with tile.TileContext(nc) as tc:
        _body(nc, tc, x, wqkv, wout, bout, y)
    nc.compile()
    return nc


def _mm(nc, out, lhsT, rhs, start, stop, width):
    """Accumulating matmul split into <=512-wide moving chunks."""
    for c0 in range(0, width, MMF):
        nc.tensor.matmul(out[:, c0:c0 + MMF], lhsT, rhs[:, c0:c0 + MMF],
                         start=start, stop=stop)


def _body(nc, tc, x, wqkv, wout, bout, y):
    # ---- persistent SBUF residents (right side) --------------------------
    wop = tc.alloc_tile_pool(name="wo", bufs=1, side="right")
    wo = [wop.tile([P, E], BF16, name=f"wo{e}", tag=f"wo{e}") for e in range(ET)]
    bo_b = wop.tile([P, E], F32, name="bo_b", tag="bo_b")

    kqp = tc.alloc_tile_pool(name="kq", bufs=1, side="right")
    kT = [kqp.tile([P, N], BF16, name=f"kT{e}", tag=f"kT{e}") for e in range(ET)]
    qT = [kqp.tile([P, NQ], BF16, name=f"qT{e}", tag=f"qT{e}") for e in range(ET)]

    vp = tc.alloc_tile_pool(name="vp", bufs=1, side="right")
    v = [vp.tile([P, E], BF16, name=f"v{m}", tag=f"v{m}") for m in range(MT)]

    smp = tc.alloc_tile_pool(name="small", bufs=1, side="right")
    ones = smp.tile([P, 1], BF16, name="ones", tag="ones")
    sums_acc = smp.tile([P, QT], F32, name="sums_acc", tag="sums_acc")
    recip = smp.tile([P, QT], F32, name="recip", tag="recip")
    actwarm = smp.tile([P, 1], F32, name="actwarm", tag="actwarm")

    # The one PSUM pool: 4 rotating [128,1024] f32 slots = all 8 banks.
    workp = tc.alloc_tile_pool(name="work", bufs=1, space="PSUM")

    def pstile(tag):
        return workp.tile([P, NQ], F32, name=tag, tag=tag)

    # ---- phase 1: load + project ----------------------------------------
    wp = tc.alloc_tile_pool(name="wqkv", bufs=1)
    wk = [wp.tile([P, E], BF16, name=f"wk{f}", tag=f"wk{f}") for f in range(FT)]
    wq = [wp.tile([P, E], BF16, name=f"wq{f}", tag=f"wq{f}") for f in range(FT)]
    wv = [wp.tile([P, E], BF16, name=f"wv{f}", tag=f"wv{f}") for f in range(FT)]

    xTp = tc.alloc_tile_pool(name="xT", bufs=1)
    xT = [[xTp.tile([P, HKEY], BF16, name=f"xT{f}_{h}", tag=f"xT{f}_{h}")
           for h in range(2)] for f in range(FT)]

    for f in range(4):
        nc.sync.dma_start(out=wk[f], in_=wqkv[f * P:(f + 1) * P, E:2 * E])
    for f in range(FT):
        nc.sync.dma_start_transpose(
            out=xT[f][0], in_=x[0:HKEY, f * P:(f + 1) * P])
    for f in range(4, FT):
        nc.sync.dma_start(out=wk[f], in_=wqkv[f * P:(f + 1) * P, E:2 * E])
    for f in range(FT):
        nc.sync.dma_start_transpose(
            out=xT[f][1], in_=x[HKEY:N, f * P:(f + 1) * P])
    for f in range(FT):
        nc.sync.dma_start(out=wq[f], in_=wqkv[f * P:(f + 1) * P, 0:E])
    for f in range(FT):
        nc.sync.dma_start(out=wv[f], in_=wqkv[f * P:(f + 1) * P, 2 * E:3 * E])
    for e in range(ET):
        nc.sync.dma_start(out=wo[e], in_=wout[e * P:(e + 1) * P, :])
    bout_bcast = bass.AP(tensor=bout.tensor, offset=0, ap=[[0, P], [1, E]])
    nc.sync.dma_start(out=bo_b, in_=bout_bcast)
    nc.vector.memset(ones, 1.0)
    # Preload the Exp act-table while ACT is idle (first real Exp would
    # otherwise pay the 1.3us LoadActFuncSet at the scores boundary).
    nc.scalar.activation(actwarm, ones, EXP, scale=0.125)

    # Projection chunk i uses PSUM slot ps[(i+1)%4]: the slot order is
    # phased so the first three scores tiles land on slots freed >=2
    # chunks before the projection ends.
    pjtag = lambda i: f"ps{(i + 1) % 4}"
    ci = 0

    # Warmup: K chunks e=0..3 of keys-half 0, f-outer: each round f only
    # needs (wk[f], xT[f][0]), which stream in one pair per ~0.9us.
    warm = [pstile(pjtag(i)) for i in range(4)]
    ci = 4
    for f in range(FT):
        for e in range(4):
            _mm(nc, warm[e], wk[f][:, e * P:(e + 1) * P], xT[f][0],
                start=(f == 0), stop=(f == FT - 1), width=HKEY)
    for e in range(4):
        nc.vector.tensor_copy(kT[e][:, 0:HKEY], warm[e])

    for (e, h) in [(e, h) for h in range(2) for e in range(ET)
                   if not (h == 0 and e < 4)]:
        ps = pstile(pjtag(ci)); ci += 1
        for f in range(FT):
            _mm(nc, ps, wk[f][:, e * P:(e + 1) * P], xT[f][h],
                start=(f == 0), stop=(f == FT - 1), width=HKEY)
        nc.vector.tensor_copy(kT[e][:, h * HKEY:(h + 1) * HKEY], ps)

    for e in range(ET):
        ps = pstile(pjtag(ci)); ci += 1
        for f in range(FT):
            _mm(nc, ps, wq[f][:, e * P:(e + 1) * P], xT[f][0],
                start=(f == 0), stop=(f == FT - 1), width=NQ)
        nc.vector.tensor_copy(qT[e], ps)

    for m in range(MT):
        h, mm_ = divmod(m, ET)
        ps = pstile(pjtag(ci)); ci += 1
        for f in range(FT):
            _mm(nc, ps, xT[f][h][:, mm_ * P:(mm_ + 1) * P], wv[f],
                start=(f == 0), stop=(f == FT - 1), width=E)
        nc.vector.tensor_copy(v[m], ps)

    xTp.release()
    wp.release()

    # ---- phase 2: scores + exp + row-sums --------------------------------
    # s(m) rotates slots [ps3, ps0, ps1] (ps2 is the row-sums slot), so
    # s(15) -> ps3 = first slot of PV group 1 (needed 6.8us later).
    pp = tc.alloc_tile_pool(name="pp", bufs=1)
    p_tiles = [pp.tile([P, NQ], BF16, name=f"p{m}", tag=f"p{m}")
               for m in range(MT)]
    stag = ["ps3", "ps0", "ps1"]

    for m in range(MT):
        s = pstile(stag[m % 3])
        for e in range(ET):
            _mm(nc, s, kT[e][:, m * P:(m + 1) * P], qT[e],
                start=(e == 0), stop=(e == ET - 1), width=NQ)
        nc.scalar.activation(p_tiles[m], s, EXP, scale=0.125)
        # Row-sum the PREVIOUS tile's exp while this tile's S matmuls
        # occupy PE (rowsums of tile 15 happen inside PV group 0).
        if m > 0:
            _row_sums(nc, p_tiles[m - 1], pstile, ones, sums_acc,
                      first=(m == 1))

    # ---- phase 3: O^T = sum_m V[m]^T P^T[m], 4 groups of 2 e-tiles ------
    # Group slots: g0 (ps0, ps1) <- s(13), s(14): free at entry.
    #              g1 (ps3, ps2) <- s(15), sums(15): ~1us into g0.
    oTp = tc.alloc_tile_pool(name="oTp", bufs=1)
    oT = [oTp.tile([P, NQ], BF16, name=f"oT{e}", tag=f"oT{e}")
          for e in range(ET)]
    PVTAGS = [("ps0", "ps1"), ("ps3", "ps2")]
    for g in range(4):
        o_ps = [pstile(t) for t in PVTAGS[g % 2]]
        for m in range(MT):
            for j in range(2):
                e = 2 * g + j
                _mm(nc, o_ps[j], v[m][:, e * P:(e + 1) * P], p_tiles[m],
                    start=(m == 0), stop=(m == MT - 1), width=NQ)
        if g == 0:
            # Deferred finale of the softmax denominators: overlaps exp(15).
            _row_sums(nc, p_tiles[MT - 1], pstile, ones, sums_acc,
                      first=False)
            nc.vector.reciprocal(recip, sums_acc)
        h2 = NQ // 2
        for j in range(2):
            e = 2 * g + j
            nc.vector.tensor_copy(oT[e][:, 0:h2], o_ps[j][:, 0:h2])
            nc.gpsimd.tensor_copy(oT[e][:, h2:NQ], o_ps[j][:, h2:NQ])

    # ---- phase 4: y = (O^T.T W_out) * recip + b_out ----------------------
    # yps slot order (ps0, ps1, ps3, ps2): matches PV group copy order.
    ytag = ["ps0", "ps1", "ps3", "ps2"]
    with tc.tile_pool(name="ysb", bufs=3) as ysp:
        for nqt in range(QT):
            yps = pstile(ytag[nqt % 4])
            for e in range(ET):
                _mm(nc, yps, oT[e][:, nqt * P:(nqt + 1) * P], wo[e],
                    start=(e == 0), stop=(e == ET - 1), width=E)
            ysb = ysp.tile([P, E], F32, name="ysb", tag="ysb")
            nsplit = 4 if nqt == QT - 1 else 2
            w = E // nsplit
            for hh in range(nsplit):
                sl = slice(hh * w, (hh + 1) * w)
                eng = nc.vector if hh % 2 == 0 or nsplit == 2 else nc.gpsimd
                eng.scalar_tensor_tensor(
                    out=ysb[:, sl], in0=yps[:, sl],
                    scalar=recip[:, nqt:nqt + 1], in1=bo_b[:, sl],
                    op0=mybir.AluOpType.mult, op1=mybir.AluOpType.add)
                nc.sync.dma_start(out=y[nqt * P:(nqt + 1) * P, sl],
                                  in_=ysb[:, sl])

    oTp.release()
    pp.release()
    smp.release()
    vp.release()
    kqp.release()
    wop.release()
    workp.release()


def _row_sums(nc, p, pstile, ones, sums_acc, first):
    # Full-slot tile on the dedicated ps2 slot; only cols 0:QT are used.
    sums_m = pstile("ps2")
    for q in range(QT):
        nc.tensor.matmul(sums_m[:, q:q + 1], p[:, q * P:(q + 1) * P], ones,
                         start=True, stop=True)
    if first:
        nc.vector.tensor_copy(sums_acc, sums_m[:, 0:QT])
    else:
        nc.vector.tensor_tensor(out=sums_acc, in0=sums_acc,
                                in1=sums_m[:, 0:QT], op=mybir.AluOpType.add)


_NC_CACHE = None


def _get_program():
    global _NC_CACHE
    if _NC_CACHE is None:
        _NC_CACHE = build_program()
    return _NC_CACHE


def kernel(x, W_qkv, W_out, b_out):
    import ml_dtypes
    from concourse.bass_utils import run_bass_kernel_spmd

    bf16 = ml_dtypes.bfloat16
    x = np.asarray(x, dtype=np.float32).astype(bf16)
    W_qkv = np.asarray(W_qkv, dtype=np.float32).astype(bf16)
    W_out = np.asarray(W_out, dtype=np.float32).astype(bf16)
    b_out = np.asarray(b_out, dtype=np.float32)

    nc = _get_program()
    in_maps = []
    for c in range(8):
        b, half = divmod(c, 2)
        xb = x[b]
        xrot = np.ascontiguousarray(
            np.concatenate([xb[half * NQ:], xb[:half * NQ]], axis=0))
        in_maps.append({"x": xrot, "wqkv": W_qkv, "wout": W_out,
                       "bout": b_out})
    res = run_bass_kernel_spmd(nc, in_maps, list(range(8)))
    out = np.empty((B, N, E), dtype=np.float32)
    for c in range(8):
        b, half = divmod(c, 2)
        out[b, half * NQ:(half + 1) * NQ] = res.results[c]["y"]
    return out


# revision 22
# speedup vs baseline: 1.0282x; 1.0002x over previous
"""Classical self-attention (head-summed scores) on 8 trn2 NeuronCores.

Math (per batch b):
    Q = x Wq; K = x Wk; V = x Wv          (W_qkv split columns 3x1024)
    S = Q K^T / 8   (full-E contraction: heads+dims summed)
    P = softmax(S, axis=-1)
    out = (P V) W_out + b_out

Sharding: 8 cores = (4 batches) x (2 query-halves). Each core gets its
batch's x rotated so its 1024 query rows come first; keys are the full
2048 rows (key order is irrelevant to the result). K/V projections are
duplicated between the 2 cores of a batch; no collectives needed.

Design notes:
  - Inputs host-converted to bf16: halves DMA and SBUF, same PE rate
    (1.0 cycles/row) as fp32r on TRN2.
  - x^T produced by DMA-transpose (XBAR) straight from DRAM. Same-type
    DMAs are grouped: a transpose<->copy mode switch drains the DMA
    pipe, serializing the two transfers. Wk0-3 load first so the
    f-outer warmup K matmuls start on the first x^T tiles.
  - K^T, V, Q^T, P all SBUF-resident in bf16 -- no DRAM staging.
  - All matmuls emit <=512-element moving patterns (ISA limit).
  - ONE PSUM pool with 4 rotating [128,1024] slots (ps0-3) spans all
    phases; the slot schedule is chosen so no phase boundary ever
    waits on a freshly-written slot (no pool-release stalls).
  - Softmax skips max-subtraction (scores ~ N(0,16) after 1/8 scale);
    1/rowsum deferred to the output projection. Row-sums of the last
    P tile + the reciprocal are folded into the first PV group.
  - ACT runs ONLY Exp (table preloaded at t=0 -- a Copy<->Exp switch
    reloads the 1.3us act table); aux PSUM->SBUF copies go to DVE and
    Pool. Output drain is fused mul+add on DVE, split 4-way at the end.
"""

import sys

sys.path.insert(0, "/opt/trn_rl_repo")

import numpy as np

import concourse.bass as bass
import concourse.mybir as mybir
import concourse.tile as tile
from concourse import bacc

B, N, E = 4, 2048, 1024
NQ = N // 2          # query rows per core
P = 128              # partitions
FT = E // P          # 8 feature (contraction) tiles
ET = E // P          # 8 embed tiles
MT = N // P          # 16 key tiles
QT = NQ // P         # 8 query tiles
HKEY = N // 2        # 1024 keys per half (x^T transpose granularity)
MMF = 512            # max moving elements per matmul instruction
F32 = mybir.dt.float32
BF16 = mybir.dt.bfloat16
EXP = mybir.ActivationFunctionType.Exp


def build_program():
    nc = bacc.Bacc("TRN2", target_bir_lowering=False, debug=False)
    x = nc.dram_tensor("x", [N, E], BF16, kind="ExternalInput").ap()
    wqkv = nc.dram_tensor("wqkv", [E, 3 * E], BF16, kind="ExternalInput").ap()
    wout = nc.dram_tensor("wout", [E, E], BF16, kind="ExternalInput").ap()
    bout = nc.dram_tensor("bout", [E], F32, kind="ExternalInput").ap()
    y = nc.dram_tensor("y", [NQ, E], F32, kind="ExternalOutput").ap()

    with tile.TileContext(nc) as tc:
        _body(nc, tc, x, wqkv, wout, bout, y)
    nc.compile()
    return nc


def _mm(nc, out, lhsT, rhs, start, stop, width):
    """Accumulating matmul split into <=512-wide moving chunks."""
    for c0 in range(0, width, MMF):
        nc.tensor.matmul(out[:, c0:c0 + MMF], lhsT, rhs[:, c0:c0 + MMF],
                         start=start, stop=stop)


def _body(nc, tc, x, wqkv, wout, bout, y):
    # ---- persistent SBUF residents (right side) --------------------------
    wop = tc.alloc_tile_pool(name="wo", bufs=1, side="right")
    wo = [wop.tile([P, E], BF16, name=f"wo{e}", tag=f"wo{e}") for e in range(ET)]
    bo_b = wop.tile([P, E], F32, name="bo_b", tag="bo_b")

    kqp = tc.alloc_tile_pool(name="kq", bufs=1, side="right")
    kT = [kqp.tile([P, N], BF16, name=f"kT{e}", tag=f"kT{e}") for e in range(ET)]
    qT = [kqp.tile([P, NQ], BF16, name=f"qT{e}", tag=f"qT{e}") for e in range(ET)]

    vp = tc.alloc_tile_pool(name="vp", bufs=1, side="right")
    v = [vp.tile([P, E], BF16, name=f"v{m}", tag=f"v{m}") for m in range(MT)]

    smp = tc.alloc_tile_pool(name="small", bufs=1, side="right")
    ones = smp.tile([P, 1], BF16, name="ones", tag="ones")
    sums_acc = smp.tile([P, QT], F32, name="sums_acc", tag="sums_acc")
    recip = smp.tile([P, QT], F32, name="recip", tag="recip")
    actwarm = smp.tile([P, 1], F32, name="actwarm", tag="actwarm")

    # The one PSUM pool: 4 rotating [128,1024] f32 slots = all 8 banks.
    workp = tc.alloc_tile_pool(name="work", bufs=1, space="PSUM")

    def pstile(tag):
        return workp.tile([P, NQ], F32, name=tag, tag=tag)

    # ---- phase 1: load + project ----------------------------------------
    wp = tc.alloc_tile_pool(name="wqkv", bufs=1)
    wk = [wp.tile([P, E], BF16, name=f"wk{f}", tag=f"wk{f}") for f in range(FT)]
    wq = [wp.tile([P, E], BF16, name=f"wq{f}", tag=f"wq{f}") for f in range(FT)]
    wv = [wp.tile([P, E], BF16, name=f"wv{f}", tag=f"wv{f}") for f in range(FT)]

    xTp = tc.alloc_tile_pool(name="xT", bufs=1)
    xT = [[xTp.tile([P, HKEY], BF16, name=f"xT{f}_{h}", tag=f"xT{f}_{h}")
           for h in range(2)] for f in range(FT)]

    for f in range(4):
        nc.sync.dma_start(out=wk[f], in_=wqkv[f * P:(f + 1) * P, E:2 * E])
    for f in range(FT):
        nc.sync.dma_start_transpose(
            out=xT[f][0], in_=x[0:HKEY, f * P:(f + 1) * P])
    for f in range(4, FT):
        nc.sync.dma_start(out=wk[f], in_=wqkv[f * P:(f + 1) * P, E:2 * E])
    for f in range(FT):
        nc.sync.dma_start_transpose(
            out=xT[f][1], in_=x[HKEY:N, f * P:(f + 1) * P])
    for f in range(FT):
        nc.sync.dma_start(out=wq[f], in_=wqkv[f * P:(f + 1) * P, 0:E])
    for f in range(FT):
        nc.sync.dma_start(out=wv[f], in_=wqkv[f * P:(f + 1) * P, 2 * E:3 * E])
    for e in range(ET):
        nc.sync.dma_start(out=wo[e], in_=wout[e * P:(e + 1) * P, :])
    bout_bcast = bass.AP(tensor=bout.tensor, offset=0, ap=[[0, P], [1, E]])
    nc.sync.dma_start(out=bo_b, in_=bout_bcast)
    nc.vector.memset(ones, 1.0)
    # Preload the Exp act-table while ACT is idle (first real Exp would
    # otherwise pay the 1.3us LoadActFuncSet at the scores boundary).
    nc.scalar.activation(actwarm, ones, EXP, scale=0.125)

    # Projection chunk i uses PSUM slot ps[(i+1)%4]: the slot order is
    # phased so the first three scores tiles land on slots freed >=2
    # chunks before the projection ends.
    pjtag = lambda i: f"ps{(i + 1) % 4}"
    ci = 0

    # Warmup: K chunks e=0..3 of keys-half 0, f-outer: each round f only
    # needs (wk[f], xT[f][0]), which stream in one pair per ~0.9us.
    warm = [pstile(pjtag(i)) for i in range(4)]
    ci = 4
    for f in range(FT):
        for e in range(4):
            _mm(nc, warm[e], wk[f][:, e * P:(e + 1) * P], xT[f][0],
                start=(f == 0), stop=(f == FT - 1), width=HKEY)
    for e in range(4):
        nc.vector.tensor_copy(kT[e][:, 0:HKEY], warm[e])

    for (e, h) in [(e, h) for h in range(2) for e in range(ET)
                   if not (h == 0 and e < 4)]:
        ps = pstile(pjtag(ci)); ci += 1
        for f in range(FT):
            _mm(nc, ps, wk[f][:, e * P:(e + 1) * P], xT[f][h],
                start=(f == 0), stop=(f == FT - 1), width=HKEY)
        nc.vector.tensor_copy(kT[e][:, h * HKEY:(h + 1) * HKEY], ps)

    for e in range(ET):
        ps = pstile(pjtag(ci)); ci += 1
        for f in range(FT):
            _mm(nc, ps, wq[f][:, e * P:(e + 1) * P], xT[f][0],
                start=(f == 0), stop=(f == FT - 1), width=NQ)
        nc.vector.tensor_copy(qT[e], ps)

    for m in range(MT):
        h, mm_ = divmod(m, ET)
        ps = pstile(pjtag(ci)); ci += 1
        for f in range(FT):
            _mm(nc, ps, xT[f][h][:, mm_ * P:(mm_ + 1) * P], wv[f],
                start=(f == 0), stop=(f == FT - 1), width=E)
        nc.vector.tensor_copy(v[m], ps)

    xTp.release()
    wp.release()

    # ---- phase 2: scores + exp + row-sums --------------------------------
    # s(m) rotates slots [ps3, ps0, ps1] (ps2 is the row-sums slot), so
    # s(15) -> ps3 = first slot of PV group 1 (needed 6.8us later).
    pp = tc.alloc_tile_pool(name="pp", bufs=1)
    p_tiles = [pp.tile([P, NQ], BF16, name=f"p{m}", tag=f"p{m}")
               for m in range(MT)]
    stag = ["ps3", "ps0", "ps1"]

    for m in range(MT):
        s = pstile(stag[m % 3])
        for e in range(ET):
            _mm(nc, s, kT[e][:, m * P:(m + 1) * P], qT[e],
                start=(e == 0), stop=(e == ET - 1), width=NQ)
        nc.scalar.activation(p_tiles[m], s, EXP, scale=0.125)
        # Row-sum the PREVIOUS tile's exp while this tile's S matmuls
        # occupy PE (rowsums of tile 15 happen inside PV group 0).
        if m > 0:
            _row_sums(nc, p_tiles[m - 1], pstile, ones, sums_acc,
                      first=(m == 1))

    # ---- phase 3: O^T = sum_m V[m]^T P^T[m], 4 groups of 2 e-tiles ------
    # Group slots: g0 (ps0, ps1) <- s(13), s(14): free at entry.
    #              g1 (ps3, ps2) <- s(15), sums(15): ~1us into g0.
    oTp = tc.alloc_tile_pool(name="oTp", bufs=1)
    oT = [oTp.tile([P, NQ], BF16, name=f"oT{e}", tag=f"oT{e}")
          for e in range(ET)]
    PVTAGS = [("ps0", "ps1"), ("ps3", "ps2")]
    for g in range(4):
        o_ps = [pstile(t) for t in PVTAGS[g % 2]]
        for m in range(MT):
            for j in range(2):
                e = 2 * g + j
                _mm(nc, o_ps[j], v[m][:, e * P:(e + 1) * P], p_tiles[m],
                    start=(m == 0), stop=(m == MT - 1), width=NQ)
        if g == 0:
            # Deferred finale of the softmax denominators: overlaps exp(15).
            _row_sums(nc, p_tiles[MT - 1], pstile, ones, sums_acc,
                      first=False)
            nc.vector.reciprocal(recip, sums_acc)
        # DVE + ACT halves (GPSIMD cannot read PSUM). ACT's one-time
        # Copy-table load lands here, 6.8us off the critical path.
        h2 = NQ // 2
        for j in range(2):
            e = 2 * g + j
            nc.vector.tensor_copy(oT[e][:, 0:h2], o_ps[j][:, 0:h2])
            nc.scalar.copy(out=oT[e][:, h2:NQ], in_=o_ps[j][:, h2:NQ])

    # ---- phase 4: y = (O^T.T W_out) * recip + b_out ----------------------
    # yps slot order (ps0, ps1, ps3, ps2): matches PV group copy order.
    ytag = ["ps0", "ps1", "ps3", "ps2"]
    CPY = mybir.ActivationFunctionType.Copy
    with tc.tile_pool(name="ysb", bufs=3) as ysp:
        for nqt in range(QT):
            yps = pstile(ytag[nqt % 4])
            # Preload b_out*rowsum into PSUM (DVE) and accumulate on top
            # (start=False): after the final *recip this is b_out + OW*recip,
            # so the bias-add vanishes from the drain chain.
            nc.vector.tensor_scalar_mul(yps, bo_b,
                                        sums_acc[:, nqt:nqt + 1])
            for e in range(ET):
                _mm(nc, yps, oT[e][:, nqt * P:(nqt + 1) * P], wo[e],
                    start=False, stop=(e == ET - 1), width=E)
            ysb = ysp.tile([P, E], F32, name="ysb", tag="ysb")
            nsplit = 4 if nqt == QT - 1 else 2
            w = E // nsplit
            for hh in range(nsplit):
                sl = slice(hh * w, (hh + 1) * w)
                if hh % 2 == 0 or nsplit == 2:
                    nc.vector.tensor_scalar_mul(ysb[:, sl], yps[:, sl],
                                                recip[:, nqt:nqt + 1])
                else:
                    nc.scalar.activation(ysb[:, sl], yps[:, sl], CPY,
                                         scale=recip[:, nqt:nqt + 1])
                nc.sync.dma_start(out=y[nqt * P:(nqt + 1) * P, sl],
                                  in_=ysb[:, sl])

    oTp.release()
    pp.release()
    smp.release()
    vp.release()
    kqp.release()
    wop.release()
    workp.release()


def _row_sums(nc, p, pstile, ones, sums_acc, first):
    # Full-slot tile on the dedicated ps2 slot; only cols 0:QT are used.
    sums_m = pstile("ps2")
    for q in range(QT):
        nc.tensor.matmul(sums_m[:, q:q + 1], p[:, q * P:(q + 1) * P], ones,
                         start=True, stop=True)
    if first:
        nc.vector.tensor_copy(sums_acc, sums_m[:, 0:QT])
    else:
        nc.vector.tensor_tensor(out=sums_acc, in0=sums_acc,
                                in1=sums_m[:, 0:QT], op=mybir.AluOpType.add)


_NC_CACHE = None


def _get_program():
    global _NC_CACHE
    if _NC_CACHE is None:
        _NC_CACHE = build_program()
    return _NC_CACHE


def kernel(x, W_qkv, W_out, b_out):
    import ml_dtypes
    from concourse.bass_utils import run_bass_kernel_spmd

    bf16 = ml_dtypes.bfloat16
    x = np.asarray(x, dtype=np.float32).astype(bf16)
    W_qkv = np.asarray(W_qkv, dtype=np.float32).astype(bf16)
    W_out = np.asarray(W_out, dtype=np.float32).astype(bf16)
    b_out = np.asarray(b_out, dtype=np.float32)

    nc = _get_program()
    in_maps = []
    for c in range(8):
        b, half = divmod(c, 2)
        xb = x[b]
        xrot = np.ascontiguousarray(
            np.concatenate([xb[half * NQ:], xb[:half * NQ]], axis=0))
        in_maps.append({"x": xrot, "wqkv": W_qkv, "wout": W_out,
                       "bout": b_out})
    res = run_bass_kernel_spmd(nc, in_maps, list(range(8)))
    out = np.empty((B, N, E), dtype=np.float32)
    for c in range(8):
        b, half = divmod(c, 2)
        out[b, half * NQ:(half + 1) * NQ] = res.results[c]["y"]
    return out


# revision 25
# speedup vs baseline: 1.0405x; 1.0120x over previous
"""Classical self-attention (head-summed scores) on 8 trn2 NeuronCores.

Math (per batch b):
    Q = x Wq; K = x Wk; V = x Wv          (W_qkv split columns 3x1024)
    S = Q K^T / 8   (full-E contraction: heads+dims summed)
    P = softmax(S, axis=-1)
    out = (P V) W_out + b_out

Sharding: 8 cores = (4 batches) x (2 query-halves). Each core gets its
batch's x rotated so its 1024 query rows come first; keys are the full
2048 rows (key order is irrelevant to the result). K/V projections are
duplicated between the 2 cores of a batch; no collectives needed.

Design notes:
  - Inputs host-converted to bf16: halves DMA and SBUF, same PE rate
    (1.0 cycles/row) as fp32r on TRN2.
  - x^T produced by DMA-transpose (XBAR) straight from DRAM. Same-type
    DMAs are grouped: a transpose<->copy mode switch drains the DMA
    pipe, serializing the two transfers. Wk0-3 load first so the
    f-outer warmup K matmuls start on the first x^T tiles.
  - K^T, V, Q^T, P all SBUF-resident in bf16 -- no DRAM staging.
  - All matmuls emit <=512-element moving patterns (ISA limit).
  - ONE PSUM pool with 4 rotating [128,1024] slots (ps0-3) spans all
    phases; the slot schedule is chosen so no phase boundary ever
    waits on a freshly-written slot (no pool-release stalls).
  - Softmax skips max-subtraction (scores ~ N(0,16) after 1/8 scale);
    1/rowsum deferred to the output projection. Row-sums of the last
    P tile + the reciprocal are folded into the first PV group.
  - ACT runs ONLY Exp (table preloaded at t=0 -- a Copy<->Exp switch
    reloads the 1.3us act table); aux PSUM->SBUF copies go to DVE and
    Pool. Output drain is fused mul+add on DVE, split 4-way at the end.
"""

import sys

sys.path.insert(0, "/opt/trn_rl_repo")

import numpy as np

import concourse.bass as bass
import concourse.mybir as mybir
import concourse.tile as tile
from concourse import bacc

B, N, E = 4, 2048, 1024
NQ = N // 2          # query rows per core
P = 128              # partitions
FT = E // P          # 8 feature (contraction) tiles
ET = E // P          # 8 embed tiles
MT = N // P          # 16 key tiles
QT = NQ // P         # 8 query tiles
HKEY = N // 2        # 1024 keys per half (x^T transpose granularity)
MMF = 512            # max moving elements per matmul instruction
F32 = mybir.dt.float32
BF16 = mybir.dt.bfloat16
EXP = mybir.ActivationFunctionType.Exp


def build_program():
    nc = bacc.Bacc("TRN2", target_bir_lowering=False, debug=False)
    x = nc.dram_tensor("x", [N, E], BF16, kind="ExternalInput").ap()
    wqkv = nc.dram_tensor("wqkv", [E, 3 * E], BF16, kind="ExternalInput").ap()
    wout = nc.dram_tensor("wout", [E, E], BF16, kind="ExternalInput").ap()
    bout = nc.dram_tensor("bout", [E], F32, kind="ExternalInput").ap()
    y = nc.dram_tensor("y", [NQ, E], F32, kind="ExternalOutput").ap()

    with tile.TileContext(nc) as tc:
        _body(nc, tc, x, wqkv, wout, bout, y)
    nc.compile()
    return nc


def _mm(nc, out, lhsT, rhs, start, stop, width):
    """Accumulating matmul split into <=512-wide moving chunks."""
    for c0 in range(0, width, MMF):
        nc.tensor.matmul(out[:, c0:c0 + MMF], lhsT, rhs[:, c0:c0 + MMF],
                         start=start, stop=stop)


def _body(nc, tc, x, wqkv, wout, bout, y):
    # ---- persistent SBUF residents (right side) --------------------------
    wop = tc.alloc_tile_pool(name="wo", bufs=1, side="right")
    wo = [wop.tile([P, E], BF16, name=f"wo{e}", tag=f"wo{e}") for e in range(ET)]
    bo_b = wop.tile([P, E], F32, name="bo_b", tag="bo_b")

    kqp = tc.alloc_tile_pool(name="kq", bufs=1, side="right")
    kT = [kqp.tile([P, N], BF16, name=f"kT{e}", tag=f"kT{e}") for e in range(ET)]
    qT = [kqp.tile([P, NQ], BF16, name=f"qT{e}", tag=f"qT{e}") for e in range(ET)]

    vp = tc.alloc_tile_pool(name="vp", bufs=1, side="right")
    v = [vp.tile([P, E], BF16, name=f"v{m}", tag=f"v{m}") for m in range(MT)]

    smp = tc.alloc_tile_pool(name="small", bufs=1, side="right")
    ones = smp.tile([P, 1], BF16, name="ones", tag="ones")
    sums_acc = smp.tile([P, QT], F32, name="sums_acc", tag="sums_acc")
    recip = smp.tile([P, QT], F32, name="recip", tag="recip")
    actwarm = smp.tile([P, 1], F32, name="actwarm", tag="actwarm")
    scratch = smp.tile([P, MMF], BF16, name="scratch", tag="scratch")

    # The one PSUM pool: 4 rotating [128,1024] f32 slots = all 8 banks.
    workp = tc.alloc_tile_pool(name="work", bufs=1, space="PSUM")

    def pstile(tag):
        return workp.tile([P, NQ], F32, name=tag, tag=tag)

    # ---- phase 1: load + project ----------------------------------------
    wp = tc.alloc_tile_pool(name="wqkv", bufs=1)
    wk = [wp.tile([P, E], BF16, name=f"wk{f}", tag=f"wk{f}") for f in range(FT)]
    wq = [wp.tile([P, E], BF16, name=f"wq{f}", tag=f"wq{f}") for f in range(FT)]
    wv = [wp.tile([P, E], BF16, name=f"wv{f}", tag=f"wv{f}") for f in range(FT)]

    xTp = tc.alloc_tile_pool(name="xT", bufs=1)
    xT = [[xTp.tile([P, HKEY], BF16, name=f"xT{f}_{h}", tag=f"xT{f}_{h}")
           for h in range(2)] for f in range(FT)]

    for f in range(4):
        nc.sync.dma_start(out=wk[f], in_=wqkv[f * P:(f + 1) * P, E:2 * E])
    for f in range(FT):
        nc.sync.dma_start_transpose(
            out=xT[f][0], in_=x[0:HKEY, f * P:(f + 1) * P])
    for f in range(4, FT):
        nc.sync.dma_start(out=wk[f], in_=wqkv[f * P:(f + 1) * P, E:2 * E])
    for f in range(FT):
        nc.sync.dma_start_transpose(
            out=xT[f][1], in_=x[HKEY:N, f * P:(f + 1) * P])
    for f in range(FT):
        nc.sync.dma_start(out=wq[f], in_=wqkv[f * P:(f + 1) * P, 0:E])
    for f in range(FT):
        nc.sync.dma_start(out=wv[f], in_=wqkv[f * P:(f + 1) * P, 2 * E:3 * E])
    for e in range(ET):
        nc.sync.dma_start(out=wo[e], in_=wout[e * P:(e + 1) * P, :])
    bout_bcast = bass.AP(tensor=bout.tensor, offset=0, ap=[[0, P], [1, E]])
    nc.sync.dma_start(out=bo_b, in_=bout_bcast)
    nc.vector.memset(ones, 1.0)
    # Preload the Exp act-table while ACT is idle (first real Exp would
    # otherwise pay the 1.3us LoadActFuncSet at the scores boundary).
    nc.scalar.activation(actwarm, ones, EXP, scale=0.125)
    # Warm the PE p-state during the DMA load window: dummy matmuls keep
    # the tensor engine's busy-streak alive so the real stream starts at
    # full clock instead of ramping through its first 3us. The dummy
    # PSUM slot (ps0) is the last one the warmup chunks claim.
    nc.vector.memset(scratch, 0.0)
    dps = pstile("ps0")
    for _ in range(34):
        nc.tensor.matmul(dps[:, 0:MMF], scratch[:, 0:P], scratch,
                         start=True, stop=True)

    # Projection chunk i uses PSUM slot ps[(i+1)%4]: the slot order is
    # phased so the first three scores tiles land on slots freed >=2
    # chunks before the projection ends.
    pjtag = lambda i: f"ps{(i + 1) % 4}"
    ci = 0

    # Warmup: K chunks e=0..3 of keys-half 0, f-outer: each round f only
    # needs (wk[f], xT[f][0]), which stream in one pair per ~0.9us.
    warm = [pstile(pjtag(i)) for i in range(4)]
    ci = 4
    for f in range(FT):
        for e in range(4):
            _mm(nc, warm[e], wk[f][:, e * P:(e + 1) * P], xT[f][0],
                start=(f == 0), stop=(f == FT - 1), width=HKEY)
    for e in range(4):
        nc.vector.tensor_copy(kT[e][:, 0:HKEY], warm[e])

    for (e, h) in [(e, h) for h in range(2) for e in range(ET)
                   if not (h == 0 and e < 4)]:
        ps = pstile(pjtag(ci)); ci += 1
        for f in range(FT):
            _mm(nc, ps, wk[f][:, e * P:(e + 1) * P], xT[f][h],
                start=(f == 0), stop=(f == FT - 1), width=HKEY)
        nc.vector.tensor_copy(kT[e][:, h * HKEY:(h + 1) * HKEY], ps)

    for e in range(ET):
        ps = pstile(pjtag(ci)); ci += 1
        for f in range(FT):
            _mm(nc, ps, wq[f][:, e * P:(e + 1) * P], xT[f][0],
                start=(f == 0), stop=(f == FT - 1), width=NQ)
        nc.vector.tensor_copy(qT[e], ps)

    for m in range(MT):
        h, mm_ = divmod(m, ET)
        ps = pstile(pjtag(ci)); ci += 1
        for f in range(FT):
            _mm(nc, ps, xT[f][h][:, mm_ * P:(mm_ + 1) * P], wv[f],
                start=(f == 0), stop=(f == FT - 1), width=E)
        nc.vector.tensor_copy(v[m], ps)

    xTp.release()
    wp.release()

    # ---- phase 2: scores + exp + row-sums --------------------------------
    # s(m) rotates slots [ps3, ps0, ps1] (ps2 is the row-sums slot), so
    # s(15) -> ps3 = first slot of PV group 1 (needed 6.8us later).
    pp = tc.alloc_tile_pool(name="pp", bufs=1)
    p_tiles = [pp.tile([P, NQ], BF16, name=f"p{m}", tag=f"p{m}")
               for m in range(MT)]
    stag = ["ps3", "ps0", "ps1"]

    for m in range(MT):
        s = pstile(stag[m % 3])
        for e in range(ET):
            _mm(nc, s, kT[e][:, m * P:(m + 1) * P], qT[e],
                start=(e == 0), stop=(e == ET - 1), width=NQ)
        nc.scalar.activation(p_tiles[m], s, EXP, scale=0.125)
        # Row-sum the PREVIOUS tile's exp while this tile's S matmuls
        # occupy PE (rowsums of tile 15 happen inside PV group 0).
        if m > 0:
            _row_sums(nc, p_tiles[m - 1], pstile, ones, sums_acc,
                      first=(m == 1))

    # ---- phase 3: O^T = sum_m V[m]^T P^T[m], 4 groups of 2 e-tiles ------
    # Group slots: g0 (ps0, ps1) <- s(13), s(14): free at entry.
    #              g1 (ps3, ps2) <- s(15), sums(15): ~1us into g0.
    oTp = tc.alloc_tile_pool(name="oTp", bufs=1)
    oT = [oTp.tile([P, NQ], BF16, name=f"oT{e}", tag=f"oT{e}")
          for e in range(ET)]
    PVTAGS = [("ps0", "ps1"), ("ps3", "ps2")]
    for g in range(4):
        o_ps = [pstile(t) for t in PVTAGS[g % 2]]
        for m in range(MT):
            for j in range(2):
                e = 2 * g + j
                _mm(nc, o_ps[j], v[m][:, e * P:(e + 1) * P], p_tiles[m],
                    start=(m == 0), stop=(m == MT - 1), width=NQ)
        if g == 0:
            # Deferred finale of the softmax denominators: overlaps exp(15).
            _row_sums(nc, p_tiles[MT - 1], pstile, ones, sums_acc,
                      first=False)
            nc.vector.reciprocal(recip, sums_acc)
        # DVE + ACT halves (GPSIMD cannot read PSUM). ACT's one-time
        # Copy-table load lands here, 6.8us off the critical path.
        h2 = NQ // 2
        for j in range(2):
            e = 2 * g + j
            nc.vector.tensor_copy(oT[e][:, 0:h2], o_ps[j][:, 0:h2])
            nc.scalar.copy(out=oT[e][:, h2:NQ], in_=o_ps[j][:, h2:NQ])

    # ---- phase 4: y = (O^T.T W_out) * recip + b_out ----------------------
    # yps slot order (ps0, ps1, ps3, ps2): matches PV group copy order.
    ytag = ["ps0", "ps1", "ps3", "ps2"]
    CPY = mybir.ActivationFunctionType.Copy
    with tc.tile_pool(name="ysb", bufs=3) as ysp:
        for nqt in range(QT):
            yps = pstile(ytag[nqt % 4])
            # Preload b_out*rowsum into PSUM (DVE) and accumulate on top
            # (start=False): after the final *recip this is b_out + OW*recip,
            # so the bias-add vanishes from the drain chain.
            nc.vector.tensor_scalar_mul(yps, bo_b,
                                        sums_acc[:, nqt:nqt + 1])
            for e in range(ET):
                _mm(nc, yps, oT[e][:, nqt * P:(nqt + 1) * P], wo[e],
                    start=False, stop=(e == ET - 1), width=E)
            ysb = ysp.tile([P, E], F32, name="ysb", tag="ysb")
            # Tapered pieces on the last chunk: the final DMA (and its
            # 0.9us completion-sem) covers only 128 columns.
            if nqt == QT - 1:
                pieces = [(0, 384), (384, 384), (768, 128), (896, 128)]
            else:
                pieces = [(0, 512), (512, 512)]
            for hh, (c0, w) in enumerate(pieces):
                sl = slice(c0, c0 + w)
                if hh % 2 == 0 or len(pieces) == 2:
                    nc.vector.tensor_scalar_mul(ysb[:, sl], yps[:, sl],
                                                recip[:, nqt:nqt + 1])
                else:
                    nc.scalar.activation(ysb[:, sl], yps[:, sl], CPY,
                                         scale=recip[:, nqt:nqt + 1])
                nc.sync.dma_start(out=y[nqt * P:(nqt + 1) * P, sl],
                                  in_=ysb[:, sl])

    oTp.release()
    pp.release()
    smp.release()
    vp.release()
    kqp.release()
    wop.release()
    workp.release()


def _row_sums(nc, p, pstile, ones, sums_acc, first):
    # Full-slot tile on the dedicated ps2 slot; only cols 0:QT are used.
    sums_m = pstile("ps2")
    for q in range(QT):
        nc.tensor.matmul(sums_m[:, q:q + 1], p[:, q * P:(q + 1) * P], ones,
                         start=True, stop=True)
    if first:
        nc.vector.tensor_copy(sums_acc, sums_m[:, 0:QT])
    else:
        nc.vector.tensor_tensor(out=sums_acc, in0=sums_acc,
                                in1=sums_m[:, 0:QT], op=mybir.AluOpType.add)


_NC_CACHE = None


def _get_program():
    global _NC_CACHE
    if _NC_CACHE is None:
        _NC_CACHE = build_program()
    return _NC_CACHE


def kernel(x, W_qkv, W_out, b_out):
    import ml_dtypes
    from concourse.bass_utils import run_bass_kernel_spmd

    bf16 = ml_dtypes.bfloat16
    x = np.asarray(x, dtype=np.float32).astype(bf16)
    W_qkv = np.asarray(W_qkv, dtype=np.float32).astype(bf16)
    W_out = np.asarray(W_out, dtype=np.float32).astype(bf16)
    b_out = np.asarray(b_out, dtype=np.float32)

    nc = _get_program()
    in_maps = []
    for c in range(8):
        b, half = divmod(c, 2)
        xb = x[b]
        xrot = np.ascontiguousarray(
            np.concatenate([xb[half * NQ:], xb[:half * NQ]], axis=0))
        in_maps.append({"x": xrot, "wqkv": W_qkv, "wout": W_out,
                       "bout": b_out})
    res = run_bass_kernel_spmd(nc, in_maps, list(range(8)))
    out = np.empty((B, N, E), dtype=np.float32)
    for c in range(8):
        b, half = divmod(c, 2)
        out[b, half * NQ:(half + 1) * NQ] = res.results[c]["y"]
    return out
